# revision 1
# baseline (speedup 1.0000x reference)
# Bahdanau-attention kernel for TRN2, data-parallel over batch across 8 NeuronCores.
#
# reference math (B=16, S=2048, H=1024):
#   h_proj = hidden @ W[:, :H].T                      [B, H]
#   e_proj = einsum('bsh,gh->bsg', enc, W[:, H:])     [B, S, H]
#   scores = tanh(h_proj[:,None,:] + e_proj + b)      [B, S, H]
#   logits = scores @ v                               [B, S]
#   out    = softmax(logits, -1)[:, None, :]          [B, 1, S]
#
# Per-core layout (2 batches/core):
#   All matmuls contract over h (or g), so every SBUF operand is laid out with
#   the contraction dim on partitions. Host pre-transposes/pre-tiles:
#     encT[bb, p, k, s] = enc[2i+bb, s, 128k+p]       (fp8 e4m3)
#     we[p, j, k, m] = 32 * W[128j+m, 1024+128k+p]    (fp8 e4m3, W_e^T pre-scaled)
#     wh[p, j, k, m] = W[128j+m, 128k+p]              (bf16, W_h^T for h_proj)
#     hiddenT[p, k, bb] = hidden[2i+bb, 128k+p]       (bf16)
#     bvec[p, j] = b[128j+p] (f32);  vvec[p, j, 0] = 16*v[128j+p] (fp8, lane 0 of 16)
#   On chip per (batch, s-block of 512):
#     for each g-tile j: psum[g,s] = sum_kp DoubleRow-MM(we pair, encT pair)
#       (fp8 DoubleRow: one MM contracts two k-tiles -> 4 MMs per group, ~2x rate)
#     scoresT = tanh(psum/32 + (h_projT + b)[g])   (ACT, per-partition bias, fp8 out)
#     logits_psum += DoubleRow-MM(v pair, scoresT pair)  (PE, fp8; v x16 in lane 0)
#     then per-block EXP from PSUM (no max-subtraction; logits bounded by ||v||_1),
#     accumulated partials, one reciprocal, scaled output row.

import numpy as np
import ml_dtypes

import concourse.bass as bass
import concourse.mybir as mybir
import concourse.tile as tile
from concourse import bacc
from concourse.bass_utils import run_bass_kernel_spmd
from concourse.tile_rust import add_dep_helper

B, S, H = 16, 2048, 1024
NCORES = 8
BPC = B // NCORES          # batches per core
KT = H // 128              # contraction tiles
GT = H // 128              # output (g) tiles
SBLK = 512                 # s-block (one PSUM bank of f32)
NSB = S // SBLK

BF16 = mybir.dt.bfloat16
F32 = mybir.dt.float32
FP8 = mybir.dt.float8e4
WSCALE = 32.0              # W_e pre-scaled into fp8's sweet range; undone in tanh's scale

_CACHE = {}


def _build():
    nc = bacc.Bacc("TRN2", target_bir_lowering=False, debug=False, num_devices=NCORES)

    encT_d = nc.dram_tensor("encT", [BPC, 128, KT, S], FP8, kind="ExternalInput")
    wh_d = nc.dram_tensor("wh", [128, GT, KT, 128], BF16, kind="ExternalInput")
    we_d = nc.dram_tensor("we", [128, GT, KT, 128], FP8, kind="ExternalInput")
    hiddenT_d = nc.dram_tensor("hiddenT", [128, KT, BPC], BF16, kind="ExternalInput")
    bvec_d = nc.dram_tensor("bvec", [128, GT], F32, kind="ExternalInput")
    vvec_d = nc.dram_tensor("vvec", [128, GT, 16], FP8, kind="ExternalInput")
    out_d = nc.dram_tensor("out", [BPC, S], F32, kind="ExternalOutput")

    ACT = mybir.ActivationFunctionType

    with tile.TileContext(nc) as tc:
        with (
            tc.tile_pool(name="const", bufs=1) as constp,
            tc.tile_pool(name="wp", bufs=1) as wp,
            tc.tile_pool(name="encp", bufs=1) as encp,
            tc.tile_pool(name="scp", bufs=2) as scp,
            tc.tile_pool(name="smallp", bufs=2) as smallp,
            tc.tile_pool(name="mps", bufs=3, space="PSUM") as mps,
            tc.tile_pool(name="lps", bufs=2, space="PSUM") as lps,
            tc.tile_pool(name="hps", bufs=2, space="PSUM") as hps,
        ):
            # --- tiny constants ---
            hiddenT_sb = constp.tile([128, KT, BPC], BF16, tag="hiddenT")
            nc.sync.dma_start(out=hiddenT_sb[:], in_=hiddenT_d[:])
            b_sb = constp.tile([128, GT], F32, tag="bvec")
            nc.sync.dma_start(out=b_sb[:], in_=bvec_d[:])
            v_sb = constp.tile([128, GT, 16], FP8, tag="vvec")
            nc.sync.dma_start(out=v_sb[:], in_=vvec_d[:])

            # --- weights: W_e^T in fp8 (DoubleRow main GEMM), W_h^T in bf16 ---
            we_sb = [None] * GT
            wh_sb = [None] * GT

            def load_we(j):
                t = wp.tile([128, KT, 128], FP8, name=f"we{j}", tag=f"we{j}")
                inst = nc.sync.dma_start(out=t[:], in_=we_d[:, j])
                we_sb[j] = t
                return inst

            def load_wh(j):
                t = wp.tile([128, KT, 128], BF16, name=f"wh{j}", tag=f"wh{j}")
                inst = nc.sync.dma_start(out=t[:], in_=wh_d[:, j])
                wh_sb[j] = t
                return inst

            # h_projT[j] = sum_k W_h(k,j).T @ hiddenT(k)  -> [128, BPC] per g-tile j,
            # then hb[:, j, bb] = h_projT + b (per-partition bias for the tanh).
            hb_sb = constp.tile([128, GT, BPC], F32, tag="hb")

            def hproj(j):
                hp = hps.tile([128, BPC], F32, tag="hp")
                last = None
                for k in range(KT):
                    last = nc.tensor.matmul(
                        hp[:],
                        wh_sb[j][:, k, :],
                        hiddenT_sb[:, k, :],
                        start=(k == 0),
                        stop=(k == KT - 1),
                    )
                nc.vector.tensor_scalar_add(hb_sb[:, j, :], hp[:], b_sb[:, j : j + 1])
                return last

            # DMA order tuned so the first main matmul group can start early:
            # We(j=0), Wh(j=0), enc b0 (first s-half), then alternating We/Wh columns.
            load_we(0)
            load_wh(0)
            hp0_gate = hproj(0)

            enc_sb = [
                encp.tile([128, KT, S], FP8, name=f"enc{bb}", tag=f"enc{bb}")
                for bb in range(BPC)
            ]

            def load_enc(bb, lo, size):
                sl = slice(lo, lo + size)
                return nc.sync.dma_start(
                    out=enc_sb[bb][:, :, sl], in_=encT_d[bb][:, :, sl]
                )

            # DMA waves: what the first matmul group needs streams immediately;
            # later waves are gated on compute milestones (add_dep_helper below)
            # so the critical wave gets the full HBM bandwidth.
            dma_waves = [[], [], []]
            load_enc(0, 0, SBLK)
            for j in range(1, GT):
                dma_waves[0].append(load_we(j))
                dma_waves[0].append(load_wh(j))
                hproj(j)
            dma_waves[0].append(load_enc(0, SBLK, SBLK))
            dma_waves[1].append(load_enc(0, 2 * SBLK, 2 * SBLK))
            dma_waves[1].append(load_enc(1, 0, 2 * SBLK))
            dma_waves[2].append(load_enc(1, 2 * SBLK, 2 * SBLK))

            # --- main loop ---
            # Softmax note: no max-subtraction — |logits| <= ||v||_1 * max|tanh| ~ 9.6,
            # so exp() cannot overflow in f32 and the softmax ratio is unchanged.
            gates = {}
            mm_gate = None
            for bb in range(BPC):
                exps = smallp.tile([1, S], F32, name=f"exps{bb}", tag=f"exps{bb}")
                parts = smallp.tile([1, NSB], F32, name=f"parts{bb}", tag=f"parts{bb}")
                for sb in range(NSB):
                    sl = slice(sb * SBLK, (sb + 1) * SBLK)
                    scps = []
                    for jp in range(GT // 2):
                        mp2 = [None, None]
                        for half in range(2):
                            j = 2 * jp + half
                            mp2[half] = mps.tile(
                                [128, SBLK], F32, tag="mp", name=f"mp{half}"
                            )
                            for kp in range(KT // 2):
                                mm = nc.tensor.matmul(
                                    mp2[half][:],
                                    we_sb[j][:, 2 * kp : 2 * kp + 2, :],
                                    enc_sb[bb][:, 2 * kp : 2 * kp + 2, sl],
                                    start=(kp == 0),
                                    stop=(kp == KT // 2 - 1),
                                    perf_mode=mybir.MatmulPerfMode.DoubleRow,
                                )
                                if mm_gate is None:
                                    mm_gate = mm
                        # fp8 scores, stored as a g-tile pair for the DoubleRow v-dot
                        sc2 = scp.tile(
                            [128, 2, SBLK], FP8, name=f"sc{jp}", tag=f"sc{jp}"
                        )
                        for half in range(2):
                            j = 2 * jp + half
                            act_inst = nc.scalar.activation(
                                sc2[:, half, :], mp2[half][:], ACT.Tanh,
                                bias=hb_sb[:, j, bb : bb + 1],
                                scale=1.0 / WSCALE,
                            )
                            gates[(bb, sb, j)] = act_inst
                        scps.append(sc2)
                    # v-dot: DoubleRow over g-tile pairs; v is x16 in fp8 lane 0,
                    # so logits land in psum row 0 scaled by 16 (undone in EXP)
                    lp = lps.tile([16, SBLK], F32, tag="lp")
                    for jp in range(GT // 2):
                        nc.tensor.matmul(
                            lp[:],
                            v_sb[:, 2 * jp : 2 * jp + 2, :],
                            scps[jp][:],
                            start=(jp == 0),
                            stop=(jp == GT // 2 - 1),
                            perf_mode=mybir.MatmulPerfMode.DoubleRow,
                        )
                    nc.scalar.activation(
                        exps[:, sl], lp[0:1, :], ACT.Exp,
                        accum_out=parts[:, sb : sb + 1],
                        scale=1.0 / 16.0,
                    )

                ssum = smallp.tile([1, 1], F32, tag="ssum")
                nc.vector.tensor_reduce(
                    ssum[:], parts[:], axis=mybir.AxisListType.X,
                    op=mybir.AluOpType.add,
                )
                rsum = smallp.tile([1, 1], F32, tag="rsum")
                nc.vector.reciprocal(rsum[:], ssum[:])
                outrow = smallp.tile([1, S], F32, name=f"outrow{bb}", tag=f"outrow{bb}")
                nc.vector.tensor_scalar_mul(outrow[:], exps[:], rsum[:])
                nc.sync.dma_start(out=out_d[bb : bb + 1, :], in_=outrow[:])

            # gate the later DMA waves on compute progress: wave 1 after the
            # first tanh of (b0, sb0); wave 2 after (b0, sb1) finishes.
            for inst in dma_waves[0]:
                add_dep_helper(
                    inst.ins, hp0_gate.ins, sync=True,
                    reason="dma wave 0 gated on hproj(0), ~4us before first main MM",
                )
            for inst in dma_waves[1]:
                add_dep_helper(
                    inst.ins, gates[(0, 0, 0)].ins, sync=True,
                    reason="dma wave 1 gated on first tanh",
                )
            for inst in dma_waves[2]:
                add_dep_helper(
                    inst.ins, gates[(0, 1, 7)].ins, sync=True,
                    reason="dma wave 2 gated on (b0,sb1) tanh",
                )

    nc.compile()
    return nc


def _get_nc():
    if "nc" not in _CACHE:
        _CACHE["nc"] = _build()
    return _CACHE["nc"]


def _make_in_maps(hidden, encoder_outputs, W, b, v):
    bf = ml_dtypes.bfloat16
    fp8 = ml_dtypes.float8_e4m3
    WT = np.ascontiguousarray(W.T)  # [2H, H]; WT[hin, gout]
    w_tiles = WT.reshape(2, KT, 128, GT, 128).transpose(0, 2, 3, 1, 4)  # [half, p, j, k, m]
    wh_host = np.ascontiguousarray(w_tiles[0]).astype(bf)
    we_host = np.ascontiguousarray(w_tiles[1] * WSCALE).astype(fp8)
    b_host = np.ascontiguousarray(b.reshape(GT, 128).T).astype(np.float32)
    v_host = np.zeros((128, GT, 16), dtype=fp8)
    v_host[:, :, 0] = (v.reshape(GT, 128).T * 16.0).astype(fp8)

    in_maps = []
    for i in range(NCORES):
        hs = hidden[BPC * i : BPC * (i + 1)]  # [BPC, H]
        es = encoder_outputs[BPC * i : BPC * (i + 1)]  # [BPC, S, H]
        hT = np.ascontiguousarray(
            hs.T.reshape(KT, 128, BPC).transpose(1, 0, 2)
        ).astype(bf)
        # [bb, p, k, s]: partition dim outermost so one DMA fills all k-tiles
        # of a column range with matching AP iteration order
        eT = np.ascontiguousarray(
            es.transpose(0, 2, 1).reshape(BPC, KT, 128, S).transpose(0, 2, 1, 3)
        ).astype(fp8)
        in_maps.append(
            {
                "encT": eT,
                "wh": wh_host,
                "we": we_host,
                "hiddenT": hT,
                "bvec": b_host,
                "vvec": v_host,
            }
        )
    return in_maps


def _run(in_maps, **kwargs):
    nc = _get_nc()
    try:
        return run_bass_kernel_spmd(
            nc, in_maps, core_ids=list(range(NCORES)), **kwargs
        )
    except Exception:
        # A first execution right after NEFF load has been seen to wedge the
        # device once; it recovers after a short pause. Retry once.
        import time as _time

        _time.sleep(20)
        return run_bass_kernel_spmd(
            nc, in_maps, core_ids=list(range(NCORES)), **kwargs
        )


def kernel(hidden, encoder_outputs, W, b, v):
    hidden = np.asarray(hidden, dtype=np.float32)
    encoder_outputs = np.asarray(encoder_outputs, dtype=np.float32)
    W = np.asarray(W, dtype=np.float32)
    b = np.asarray(b, dtype=np.float32)
    v = np.asarray(v, dtype=np.float32)

    in_maps = _make_in_maps(hidden, encoder_outputs, W, b, v)
    res = _run(in_maps)
    outs = [np.asarray(res.results[i]["out"], dtype=np.float32) for i in range(NCORES)]
    return np.concatenate(outs, axis=0).reshape(B, 1, S)



# revision 18
# speedup vs baseline: 1.0318x; 1.0318x over previous
# Bahdanau-attention kernel for TRN2, data-parallel over batch across 8 NeuronCores.
#
# reference math (B=16, S=2048, H=1024):
#   h_proj = hidden @ W[:, :H].T                      [B, H]
#   e_proj = einsum('bsh,gh->bsg', enc, W[:, H:])     [B, S, H]
#   scores = tanh(h_proj[:,None,:] + e_proj + b)      [B, S, H]
#   logits = scores @ v                               [B, S]
#   out    = softmax(logits, -1)[:, None, :]          [B, 1, S]
#
# Per-core (2 batches): the e_proj GEMM is fp8 DoubleRow at the PE roofline
# (~216ns per [128x512] MM covering a 256-row contraction). Phase structure:
# one phase per (batch, s-block of 512); per phase 8 g-tiles x 4 k-pair MMs
# into single-bank PSUM slots (ring of 6), drained by one tanh each
# (fp8 scores out, per-partition bias carries h_proj + b). h_proj itself is
# fp8 DR (32 tiny MMs) interleaved into the initial DMA window. v-dots are
# deferred per batch and column-packed 4-wide via tile_position into one
# PSUM bank; softmax runs on a [4, 512] partition-distributed layout
# (strided-partition exp + tiny DMA partition gathers).
import numpy as np
import ml_dtypes

import concourse.bass as bass
import concourse.mybir as mybir
import concourse.tile as tile
from concourse import bacc
from concourse.bass_utils import run_bass_kernel_spmd
from concourse.tile_rust import add_dep_helper

B, S, H = 16, 2048, 1024
NCORES = 8
BPC = B // NCORES          # batches per core
KT = H // 128              # contraction tiles
GT = H // 128              # output (g) tiles
SBLK = 512
NSB = S // SBLK

BF16 = mybir.dt.bfloat16
F32 = mybir.dt.float32
FP8 = mybir.dt.float8e4
DR = mybir.MatmulPerfMode.DoubleRow
WSCALE = 32.0

STRIDED_EXP = False        # BIR verifier rejects partition-strided ACT reads

_CACHE = {}


def _build():
    nc = bacc.Bacc("TRN2", target_bir_lowering=False, debug=False, num_devices=NCORES)

    encT_d = nc.dram_tensor("encT", [BPC, 128, NSB, KT, SBLK], FP8, kind="ExternalInput")
    we_d = nc.dram_tensor("we", [128, GT, KT, 128], FP8, kind="ExternalInput")
    whf_d = nc.dram_tensor("whf", [128, GT, KT, 128], FP8, kind="ExternalInput")
    hiddenT_d = nc.dram_tensor("hiddenT", [128, KT, BPC], FP8, kind="ExternalInput")
    bvec_d = nc.dram_tensor("bvec", [128, GT], F32, kind="ExternalInput")
    vvec_d = nc.dram_tensor("vvec", [128, GT, 16], FP8, kind="ExternalInput")
    out_d = nc.dram_tensor("out", [BPC, S], F32, kind="ExternalOutput")

    ACT = mybir.ActivationFunctionType

    with tile.TileContext(nc) as tc:
        with (
            tc.tile_pool(name="const", bufs=1) as constp,
            tc.tile_pool(name="wp", bufs=1) as wp,
            tc.tile_pool(name="encp", bufs=1) as encp,
            tc.tile_pool(name="scp", bufs=1) as scp,
            tc.tile_pool(name="smallp", bufs=2) as smallp,
            tc.tile_pool(name="mps", bufs=4, space="PSUM") as mps,
            tc.tile_pool(name="lps", bufs=1, space="PSUM") as lps,
        ):
            # ---- ACT table preload: dummy tanh with no data deps ----
            dmy = constp.tile([1, 1], F32, tag="dmy")
            nc.vector.memset(dmy[:], 0.0)
            dmy2 = constp.tile([1, 1], F32, tag="dmy2")
            nc.scalar.activation(dmy2[:], dmy[:], ACT.Tanh)

            # ---- DMA: sync queue carries the batch-0 critical path, in
            # need-order; gpsimd (SWDGE) carries bulk batch-1 enc + consts.
            hiddenT_sb = constp.tile([128, KT, BPC], FP8, tag="hiddenT")
            nc.sync.dma_start(out=hiddenT_sb[:], in_=hiddenT_d[:])
            whf_sb = wp.tile([128, GT, KT, 128], FP8, tag="whf")
            nc.sync.dma_start(out=whf_sb[:, 0], in_=whf_d[:, 0])
            we_sb = wp.tile([128, GT, KT, 128], FP8, tag="we")
            enc_sb = [
                encp.tile([128, NSB, KT, SBLK], FP8, name=f"enc{bb}", tag=f"enc{bb}")
                for bb in range(BPC)
            ]
            nc.sync.dma_start(out=we_sb[:, 0:2], in_=we_d[:, 0:2])
            nc.sync.dma_start(out=enc_sb[0][:, 0], in_=encT_d[0][:, 0])
            nc.sync.dma_start(out=whf_sb[:, 1:4], in_=whf_d[:, 1:4])
            nc.sync.dma_start(out=we_sb[:, 2:4], in_=we_d[:, 2:4])
            nc.sync.dma_start(out=enc_sb[0][:, 1], in_=encT_d[0][:, 1])
            nc.sync.dma_start(out=whf_sb[:, 4:8], in_=whf_d[:, 4:8])
            nc.sync.dma_start(out=we_sb[:, 4:8], in_=we_d[:, 4:8])
            nc.sync.dma_start(out=enc_sb[0][:, 2], in_=encT_d[0][:, 2])
            nc.sync.dma_start(out=enc_sb[0][:, 3], in_=encT_d[0][:, 3])

            b_sb = constp.tile([128, GT], F32, tag="bvec")
            nc.gpsimd.dma_start(out=b_sb[:], in_=bvec_d[:])
            v_sb = constp.tile([128, GT, 16], FP8, tag="vvec")
            nc.gpsimd.dma_start(out=v_sb[:], in_=vvec_d[:])
            # batch-1 enc: gated below on compute milestones so the batch-0
            # critical DMAs get full HBM bandwidth first.
            encb1_dmas = [
                nc.gpsimd.dma_start(out=enc_sb[1][:, 0:2], in_=encT_d[1][:, 0:2]),
                nc.gpsimd.dma_start(out=enc_sb[1][:, 2:4], in_=encT_d[1][:, 2:4]),
            ]

            # ---- h_proj (fp8 DR, g on partitions) + bias ----
            # hp lives in the lp tag: its readers (DVE bias ops) finish in
            # phase 0, long before lp_b1 cycles back into its slot.
            hp = lps.tile([128, GT, BPC], F32, tag="lp", name="hp")
            hb_sb = constp.tile([128, GT, BPC], F32, tag="hb")

            def hproj(j):
                for kp in range(KT // 2):
                    nc.tensor.matmul(
                        hp[:, j, :],
                        whf_sb[:, j, 2 * kp : 2 * kp + 2, :],
                        hiddenT_sb[:, 2 * kp : 2 * kp + 2, :],
                        start=(kp == 0),
                        stop=(kp == KT // 2 - 1),
                        perf_mode=DR,
                    )
                nc.vector.tensor_scalar(
                    hb_sb[:, j, :], hp[:, j, :],
                    1.0 / WSCALE, b_sb[:, j : j + 1],
                    mybir.AluOpType.mult, mybir.AluOpType.add,
                )

            # scores, fp8, [p, sb, j, s']
            sc_sb = [
                scp.tile([128, NSB, GT, SBLK], FP8, name=f"sc{bb}", tag=f"sc{bb}")
                for bb in range(BPC)
            ]
            # softmax epilogue state, all on partition 0 (engine APs must be
            # 32-aligned in partition base, so spreading over partitions 0..3
            # is not expressible)
            exps_row = [
                smallp.tile([1, NSB, SBLK], F32, name=f"exps{bb}", tag=f"exps{bb}")
                for bb in range(BPC)
            ]
            rsum1 = [
                smallp.tile([1, 1], F32, name=f"rsum{bb}", tag=f"rsum{bb}")
                for bb in range(BPC)
            ]
            outrow = [
                smallp.tile([1, NSB, SBLK], F32, name=f"outrow{bb}", tag=f"outrow{bb}")
                for bb in range(BPC)
            ]

            # hproj j0/j1 fill the initial DMA window; rest interleave below
            hproj(0)
            hproj(1)

            tanh_insts = {}

            def main_phase(h, sb, interleave_hproj):
                for j in range(GT):
                    mp = mps.tile([128, SBLK], F32, tag="mp", name=f"mp{h}{sb}{j}")
                    for kp in range(KT // 2):
                        nc.tensor.matmul(
                            mp[:],
                            we_sb[:, j, 2 * kp : 2 * kp + 2, :],
                            enc_sb[h][:, sb, 2 * kp : 2 * kp + 2, :],
                            start=(kp == 0),
                            stop=(kp == KT // 2 - 1),
                            perf_mode=DR,
                        )
                    if interleave_hproj and j + 2 < GT:
                        hproj(j + 2)
                    tanh_insts[(h, sb, j)] = nc.scalar.activation(
                        sc_sb[h][:, sb, j, :], mp[:], ACT.Tanh,
                        bias=hb_sb[:, j, h : h + 1],
                        scale=1.0 / WSCALE,
                    )

            def vdot_batch(h):
                # logits for the 4 s-blocks land on partition rows 0 of 4
                # consecutive PSUM banks (one [16,512] slice per bank)
                lp = lps.tile([16, NSB, SBLK], F32, tag="lp", name=f"lp{h}")
                for sb in range(NSB):
                    for jp in range(GT // 2):
                        nc.tensor.matmul(
                            lp[:, sb, :],
                            v_sb[:, 2 * jp : 2 * jp + 2, :],
                            sc_sb[h][:, sb, 2 * jp : 2 * jp + 2, :],
                            start=(jp == 0),
                            stop=(jp == GT // 2 - 1),
                            perf_mode=DR,
                        )
                # one exp over all 2048 logits (multi-bank read, row 0),
                # with the softmax denominator accumulated inline
                ssum = smallp.tile([1, 1], F32, tag=f"ssum{h}", name=f"ssum{h}")
                nc.scalar.activation(
                    exps_row[h][:, :, :],
                    lp[0:1, :, :],
                    ACT.Exp,
                    scale=1.0 / 16.0,
                    accum_out=ssum[:],
                )
                nc.vector.reciprocal(rsum1[h][:], ssum[:])
                nc.vector.tensor_scalar_mul(
                    outrow[h][:], exps_row[h][:], rsum1[h][:, 0:1]
                )
                nc.sync.dma_start(out=out_d[h : h + 1, :], in_=outrow[h][:])

            # ---- phases ----
            for sb in range(NSB):
                main_phase(0, sb, interleave_hproj=(sb == 0))
            vdot_batch(0)
            for sb in range(NSB):
                main_phase(1, sb, interleave_hproj=False)
            vdot_batch(1)

            # gate batch-1 enc DMAs on batch-0 compute progress so the
            # critical batch-0 stream gets full HBM bandwidth first
            add_dep_helper(
                encb1_dmas[0].ins, tanh_insts[(0, 0, 0)].ins, sync=True,
                reason="enc b1 first half after phase(0,0) starts draining",
            )
            add_dep_helper(
                encb1_dmas[1].ins, tanh_insts[(0, 1, 0)].ins, sync=True,
                reason="enc b1 second half after phase(0,1) starts draining",
            )

    nc.compile()
    return nc


def _get_nc():
    if "nc" not in _CACHE:
        _CACHE["nc"] = _build()
    return _CACHE["nc"]


def _make_in_maps(hidden, encoder_outputs, W, b, v):
    fp8 = ml_dtypes.float8_e4m3
    WT = np.ascontiguousarray(W.T)  # [2H, H]; WT[hin, gout]
    w_tiles = WT.reshape(2, KT, 128, GT, 128).transpose(0, 2, 3, 1, 4)  # [half, p, j, k, m]
    whf_host = np.ascontiguousarray(w_tiles[0] * WSCALE).astype(fp8)
    we_host = np.ascontiguousarray(w_tiles[1] * WSCALE).astype(fp8)
    b_host = np.ascontiguousarray(b.reshape(GT, 128).T).astype(np.float32)
    v_host = np.zeros((128, GT, 16), dtype=fp8)
    v_host[:, :, 0] = (v.reshape(GT, 128).T * 16.0).astype(fp8)

    in_maps = []
    for i in range(NCORES):
        hs = hidden[BPC * i : BPC * (i + 1)]  # [BPC, H]
        es = encoder_outputs[BPC * i : BPC * (i + 1)]  # [BPC, S, H]
        hT = np.ascontiguousarray(
            hs.T.reshape(KT, 128, BPC).transpose(1, 0, 2)
        ).astype(fp8)
        # encT[bb, p, sb, k, s'] = enc[bb, sb*512+s', 128k+p]
        eT = np.ascontiguousarray(
            es.reshape(BPC, NSB, SBLK, KT, 128).transpose(0, 4, 1, 3, 2)
        ).astype(fp8)
        in_maps.append(
            {
                "encT": eT,
                "we": we_host,
                "whf": whf_host,
                "hiddenT": hT,
                "bvec": b_host,
                "vvec": v_host,
            }
        )
    return in_maps


def _run(in_maps, **kwargs):
    nc = _get_nc()
    try:
        return run_bass_kernel_spmd(
            nc, in_maps, core_ids=list(range(NCORES)), **kwargs
        )
    except Exception:
        import time as _time

        _time.sleep(20)
        return run_bass_kernel_spmd(
            nc, in_maps, core_ids=list(range(NCORES)), **kwargs
        )


def kernel(hidden, encoder_outputs, W, b, v):
    hidden = np.asarray(hidden, dtype=np.float32)
    encoder_outputs = np.asarray(encoder_outputs, dtype=np.float32)
    W = np.asarray(W, dtype=np.float32)
    b = np.asarray(b, dtype=np.float32)
    v = np.asarray(v, dtype=np.float32)

    in_maps = _make_in_maps(hidden, encoder_outputs, W, b, v)
    res = _run(in_maps)
    outs = [np.asarray(res.results[i]["out"], dtype=np.float32) for i in range(NCORES)]
    return np.concatenate(outs, axis=0).reshape(B, 1, S)


# revision 25
# speedup vs baseline: 1.0533x; 1.0208x over previous
# Bahdanau-attention kernel for TRN2, data-parallel over batch across 8 NeuronCores.
#
# reference math (B=16, S=2048, H=1024):
#   h_proj = hidden @ W[:, :H].T                      [B, H]
#   e_proj = einsum('bsh,gh->bsg', enc, W[:, H:])     [B, S, H]
#   scores = tanh(h_proj[:,None,:] + e_proj + b)      [B, S, H]
#   logits = scores @ v                               [B, S]
#   out    = softmax(logits, -1)[:, None, :]          [B, 1, S]
#
# Per-core (2 batches): the e_proj GEMM is fp8 DoubleRow at the PE roofline
# (~216ns per [128x512] MM covering a 256-row contraction). Phase structure:
# one phase per (batch, s-block of 512); per phase 8 g-tiles x 4 k-pair MMs
# into single-bank PSUM slots (ring of 6), drained by one tanh each
# (fp8 scores out, per-partition bias carries h_proj + b). h_proj itself is
# fp8 DR (32 tiny MMs) interleaved into the initial DMA window. v-dots are
# deferred per batch and column-packed 4-wide via tile_position into one
# PSUM bank; softmax runs on a [4, 512] partition-distributed layout
# (strided-partition exp + tiny DMA partition gathers).
import numpy as np
import ml_dtypes

import concourse.bass as bass
import concourse.mybir as mybir
import concourse.tile as tile
from concourse import bacc
from concourse.bass_utils import run_bass_kernel_spmd
from concourse.tile_rust import add_dep_helper

B, S, H = 16, 2048, 1024
NCORES = 8
BPC = B // NCORES          # batches per core
KT = H // 128              # contraction tiles
GT = H // 128              # output (g) tiles
SBLK = 512
NSB = S // SBLK

BF16 = mybir.dt.bfloat16
F32 = mybir.dt.float32
FP8 = mybir.dt.float8e4
DR = mybir.MatmulPerfMode.DoubleRow
WSCALE = 32.0

STRIDED_EXP = False        # BIR verifier rejects partition-strided ACT reads

_CACHE = {}


def _build():
    nc = bacc.Bacc("TRN2", target_bir_lowering=False, debug=False, num_devices=NCORES)

    encT_d = nc.dram_tensor("encT", [BPC, 128, NSB, KT, SBLK], FP8, kind="ExternalInput")
    we_d = nc.dram_tensor("we", [128, GT, KT, 128], FP8, kind="ExternalInput")
    whf_d = nc.dram_tensor("whf", [128, GT, KT, 128], FP8, kind="ExternalInput")
    hiddenT_d = nc.dram_tensor("hiddenT", [128, KT, BPC], FP8, kind="ExternalInput")
    bvec_d = nc.dram_tensor("bvec", [128, GT], F32, kind="ExternalInput")
    vvec_d = nc.dram_tensor("vvec", [128, GT, 16], FP8, kind="ExternalInput")
    out_d = nc.dram_tensor("out", [BPC, S], F32, kind="ExternalOutput")

    ACT = mybir.ActivationFunctionType

    with tile.TileContext(nc) as tc:
        with (
            tc.tile_pool(name="const", bufs=1) as constp,
            tc.tile_pool(name="wp", bufs=1) as wp,
            tc.tile_pool(name="encp", bufs=1) as encp,
            tc.tile_pool(name="scp", bufs=1) as scp,
            tc.tile_pool(name="smallp", bufs=2) as smallp,
            tc.tile_pool(name="mps", bufs=7, space="PSUM") as mps,
            tc.tile_pool(name="lps", bufs=1, space="PSUM") as lps,
        ):
            # ---- ACT table preload: dummy tanh with no data deps ----
            dmy = constp.tile([1, 1], F32, tag="dmy")
            nc.vector.memset(dmy[:], 0.0)
            dmy2 = constp.tile([1, 1], F32, tag="dmy2")
            nc.scalar.activation(dmy2[:], dmy[:], ACT.Tanh)

            # ---- DMA: sync queue carries the batch-0 critical path, in
            # need-order; gpsimd (SWDGE) carries bulk batch-1 enc + consts.
            hiddenT_sb = constp.tile([128, KT, BPC], FP8, tag="hiddenT")
            nc.sync.dma_start(out=hiddenT_sb[:], in_=hiddenT_d[:])
            whf_sb = wp.tile([128, GT, KT, 128], FP8, tag="whf")
            nc.sync.dma_start(out=whf_sb[:, 0:2], in_=whf_d[:, 0:2])
            we_sb = wp.tile([128, GT, KT, 128], FP8, tag="we")
            enc_sb = [
                encp.tile([128, NSB, KT, SBLK], FP8, name=f"enc{bb}", tag=f"enc{bb}")
                for bb in range(BPC)
            ]
            nc.sync.dma_start(out=whf_sb[:, 2:8], in_=whf_d[:, 2:8])
            nc.sync.dma_start(out=we_sb[:, 0:2], in_=we_d[:, 0:2])
            nc.sync.dma_start(out=enc_sb[0][:, 0], in_=encT_d[0][:, 0])
            nc.sync.dma_start(out=we_sb[:, 2:4], in_=we_d[:, 2:4])
            nc.sync.dma_start(out=enc_sb[0][:, 1], in_=encT_d[0][:, 1])
            nc.sync.dma_start(out=we_sb[:, 4:8], in_=we_d[:, 4:8])
            nc.sync.dma_start(out=enc_sb[0][:, 2], in_=encT_d[0][:, 2])
            nc.sync.dma_start(out=enc_sb[0][:, 3], in_=encT_d[0][:, 3])

            b_sb = constp.tile([128, GT], F32, tag="bvec")
            nc.gpsimd.dma_start(out=b_sb[:], in_=bvec_d[:])
            v_sb = constp.tile([128, GT, 16], FP8, tag="vvec")
            nc.gpsimd.dma_start(out=v_sb[:], in_=vvec_d[:])
            # batch-1 enc: gated below on compute milestones so the batch-0
            # critical DMAs get full HBM bandwidth first.
            encb1_dmas = [
                nc.gpsimd.dma_start(out=enc_sb[1][:, 0:2], in_=encT_d[1][:, 0:2]),
                nc.gpsimd.dma_start(out=enc_sb[1][:, 2:4], in_=encT_d[1][:, 2:4]),
            ]

            # ---- h_proj (fp8 DR, g on partitions) + bias ----
            # hp lives in the lp tag: its readers (DVE bias ops) finish in
            # phase 0, long before lp_b1 cycles back into its slot.
            hp = lps.tile([128, GT, BPC], F32, tag="lp", name="hp")
            hb_sb = constp.tile([128, GT, BPC], F32, tag="hb")

            def hproj(j):
                for kp in range(KT // 2):
                    nc.tensor.matmul(
                        hp[:, j, :],
                        whf_sb[:, j, 2 * kp : 2 * kp + 2, :],
                        hiddenT_sb[:, 2 * kp : 2 * kp + 2, :],
                        start=(kp == 0),
                        stop=(kp == KT // 2 - 1),
                        perf_mode=DR,
                    )
                nc.vector.tensor_scalar(
                    hb_sb[:, j, :], hp[:, j, :],
                    1.0 / WSCALE, b_sb[:, j : j + 1],
                    mybir.AluOpType.mult, mybir.AluOpType.add,
                )

            # scores, fp8, [p, sb, j, s']
            sc_sb = [
                scp.tile([128, NSB, GT, SBLK], FP8, name=f"sc{bb}", tag=f"sc{bb}")
                for bb in range(BPC)
            ]
            # softmax epilogue state, all on partition 0 (engine APs must be
            # 32-aligned in partition base, so spreading over partitions 0..3
            # is not expressible)
            exps_row = [
                smallp.tile([1, NSB, SBLK], F32, name=f"exps{bb}", tag=f"exps{bb}")
                for bb in range(BPC)
            ]
            parts_row = [
                smallp.tile([1, NSB], F32, name=f"parts{bb}", tag=f"parts{bb}")
                for bb in range(BPC)
            ]
            rsum1 = [
                smallp.tile([1, 1], F32, name=f"rsum{bb}", tag=f"rsum{bb}")
                for bb in range(BPC)
            ]
            outrow = [
                smallp.tile([1, NSB, SBLK], F32, name=f"outrow{bb}", tag=f"outrow{bb}")
                for bb in range(BPC)
            ]

            # all of h_proj runs inside the initial DMA window, before the
            # first main matmul needs enc data
            for j in range(GT):
                hproj(j)

            tanh_insts = {}

            def main_phase(h, sb):
                for j in range(GT):
                    mp = mps.tile([128, SBLK], F32, tag="mp", name=f"mp{h}{sb}{j}")
                    for kp in range(KT // 2):
                        nc.tensor.matmul(
                            mp[:],
                            we_sb[:, j, 2 * kp : 2 * kp + 2, :],
                            enc_sb[h][:, sb, 2 * kp : 2 * kp + 2, :],
                            start=(kp == 0),
                            stop=(kp == KT // 2 - 1),
                            perf_mode=DR,
                        )
                    tanh_insts[(h, sb, j)] = nc.scalar.activation(
                        sc_sb[h][:, sb, j, :], mp[:], ACT.Tanh,
                        bias=hb_sb[:, j, h : h + 1],
                        scale=1.0 / WSCALE,
                    )

            def vdot_batch(h):
                # plain-fp8 v-dot, column-packed: the 4 s-blocks' MMs target
                # col-groups {0,32,64,96} of one PSUM bank and execute
                # concurrently (~4ns apart); logits land on rows {0,32,64,96}
                lp = lps.tile([128, SBLK], F32, tag="lp", name=f"lp{h}")
                for j in range(GT):
                    for sb in range(NSB):
                        nc.tensor.matmul(
                            lp[32 * sb : 32 * sb + 16, :],
                            v_sb[:, j, :],
                            sc_sb[h][:, sb, j, :],
                            start=(j == 0),
                            stop=(j == GT - 1),
                            tile_position=(0, 32 * sb),
                        )
                for sb in range(NSB):
                    nc.scalar.activation(
                        exps_row[h][:, sb, :],
                        lp[32 * sb : 32 * sb + 1, :],
                        ACT.Exp,
                        scale=1.0 / 16.0,
                        accum_out=parts_row[h][:, sb : sb + 1],
                    )
                ssum = smallp.tile([1, 1], F32, tag=f"ssum{h}", name=f"ssum{h}")
                nc.vector.tensor_reduce(
                    ssum[:], parts_row[h][:], axis=mybir.AxisListType.X,
                    op=mybir.AluOpType.add,
                )
                nc.vector.reciprocal(rsum1[h][:], ssum[:])
                nc.vector.tensor_scalar_mul(
                    outrow[h][:], exps_row[h][:], rsum1[h][:, 0:1]
                )
                nc.sync.dma_start(out=out_d[h : h + 1, :], in_=outrow[h][:])

            # ---- phases ----
            for sb in range(NSB):
                main_phase(0, sb)
            vdot_batch(0)
            for sb in range(NSB):
                main_phase(1, sb)
            vdot_batch(1)

            # gate batch-1 enc DMAs on batch-0 compute progress so the
            # critical batch-0 stream gets full HBM bandwidth first
            add_dep_helper(
                encb1_dmas[0].ins, tanh_insts[(0, 0, 0)].ins, sync=True,
                reason="enc b1 first half after phase(0,0) starts draining",
            )
            add_dep_helper(
                encb1_dmas[1].ins, tanh_insts[(0, 1, 0)].ins, sync=True,
                reason="enc b1 second half after phase(0,1) starts draining",
            )

    nc.compile()
    return nc


def _get_nc():
    if "nc" not in _CACHE:
        _CACHE["nc"] = _build()
    return _CACHE["nc"]


def _make_in_maps(hidden, encoder_outputs, W, b, v):
    fp8 = ml_dtypes.float8_e4m3
    WT = np.ascontiguousarray(W.T)  # [2H, H]; WT[hin, gout]
    w_tiles = WT.reshape(2, KT, 128, GT, 128).transpose(0, 2, 3, 1, 4)  # [half, p, j, k, m]
    whf_host = np.ascontiguousarray(w_tiles[0] * WSCALE).astype(fp8)
    we_host = np.ascontiguousarray(w_tiles[1] * WSCALE).astype(fp8)
    b_host = np.ascontiguousarray(b.reshape(GT, 128).T).astype(np.float32)
    v_host = np.zeros((128, GT, 16), dtype=fp8)
    v_host[:, :, 0] = (v.reshape(GT, 128).T * 16.0).astype(fp8)

    in_maps = []
    for i in range(NCORES):
        hs = hidden[BPC * i : BPC * (i + 1)]  # [BPC, H]
        es = encoder_outputs[BPC * i : BPC * (i + 1)]  # [BPC, S, H]
        hT = np.ascontiguousarray(
            hs.T.reshape(KT, 128, BPC).transpose(1, 0, 2)
        ).astype(fp8)
        # encT[bb, p, sb, k, s'] = enc[bb, sb*512+s', 128k+p]
        eT = np.ascontiguousarray(
            es.reshape(BPC, NSB, SBLK, KT, 128).transpose(0, 4, 1, 3, 2)
        ).astype(fp8)
        in_maps.append(
            {
                "encT": eT,
                "we": we_host,
                "whf": whf_host,
                "hiddenT": hT,
                "bvec": b_host,
                "vvec": v_host,
            }
        )
    return in_maps


def _run(in_maps, **kwargs):
    nc = _get_nc()
    try:
        return run_bass_kernel_spmd(
            nc, in_maps, core_ids=list(range(NCORES)), **kwargs
        )
    except Exception:
        import time as _time

        _time.sleep(20)
        return run_bass_kernel_spmd(
            nc, in_maps, core_ids=list(range(NCORES)), **kwargs
        )


def kernel(hidden, encoder_outputs, W, b, v):
    hidden = np.asarray(hidden, dtype=np.float32)
    encoder_outputs = np.asarray(encoder_outputs, dtype=np.float32)
    W = np.asarray(W, dtype=np.float32)
    b = np.asarray(b, dtype=np.float32)
    v = np.asarray(v, dtype=np.float32)

    in_maps = _make_in_maps(hidden, encoder_outputs, W, b, v)
    res = _run(in_maps)
    outs = [np.asarray(res.results[i]["out"], dtype=np.float32) for i in range(NCORES)]
    return np.concatenate(outs, axis=0).reshape(B, 1, S)
